# revision 30
# baseline (speedup 1.0000x reference)
"""DCT-based 1.25x upsample (2D DCT-II -> zero-pad spectrum -> 2D IDCT).

The reference computation is linear per (b, c) slice: out = M @ x @ M^T with
M = E960[:, :768] @ D768 (960x768). M is *centrosymmetric*
(M[959-i, 767-n] = M[i, n]), so the symmetric/antisymmetric fold halves the
matmul FLOPs:

    MP = (M[:480, :384] + M[:480, 767:383:-1]) / 2     [480, 384]
    MM = (M[:480, :384] - M[:480, 767:383:-1]) / 2
    x_pq = (row-fold p)(col-fold q)(x)                 4 tiles of [384, 384]

    P1 = x_pp MP^T   P2 = x_pm MM^T   P3 = x_mp MP^T   P4 = x_mm MM^T
    A = P1+P2  C = P1-P2  B = P3+P4  D = P3-P4         [384, 480] each
    Q1 = MP A   Q2 = MM B   Q3 = MP C   Q4 = MM D      [480, 480] each
    out quadrants = (Q1 +- Q2, Q3 +- Q4) with flips    (done on HOST)

Three further folds, all with the O(N^2) pre/post work on the host:

1. Level-2 row fold (MP centrosymmetric): x_pp/x_pm are row-permuted into
   folded order so A/C come out of stage 1 row-folded, and Q1/Q3 contract
   K=192 per output half (mt2a/mt2b constants). The two 64-row kl1 tails
   are issued back-to-back at base partitions 0/64 -> disjoint PE
   row-groups, streaming concurrently (one 480-cycle slot for both).
2. j-fold (stage-1 output basis): all stage-1 products are computed in the
   column basis [P(:,jh)+P(:,479-jh) | difference]. For the MP-path
   products the fold pairs with an m-fold of the input (S[m]=S[383-m]),
   halving stage-1 t=0,2 contraction to K=192: two N=240 matmuls plus one
   merged full-row matmul whose constant rhs is block-diagonal over the
   two 64-row tails. The j columns are inert through stage 2; the host
   unfolds them at the end.
3. Row fold on Q2/Q4 for free: the j-folded MM-path weights MMj equal
   (MMrow+ | MMrow-)^T, so the same mt slots serve as stage-2 lhsT and
   Q2/Q4 come out row-folded (host unfolds).

Per-slice PE work: stage 1 ~6.1us, folded stage 2 ~2.9us, unfolded ~5.0us
(~92% of pure streaming cycles; LDWEIGHTS pipelines in the background for
full-row weights, only partial-row loads serialize ~107ns).

Schedule notes (measured on trn2):
- The HAM clock gate runs the PE at half speed until ~3.4us of sustained
  activity; any PE idle gap risks re-throttle. Warmup matmuls bridge until
  the first x chunk + MP-path constants land (~12us incl. the ~2us HBM
  write-receipt latency on DMA completion semaphores); slice-0 stage-1
  runs its three a-groups first since they need only mt slots 0:2 + x t0.
- Slice-0 x is split per product t on the sync HWDGE queue; constants are
  split MP-path-first on the scalar queue (queues drain round-robin, so
  order on each queue matches consumption). The gpsimd queue is avoided:
  SWDGE starts slow and costs a ~5us DGE drain at teardown. GpSimd compute
  is also ~2.5x slower per elementwise op than DVE - keep it idle.
- Stage-1 drain per step: ScalarE casts pb2=2*P2 (bf16), DVE computes
  A = 0.5*pb2 + ps_a (the only PSUM reads, stuck at 1x mode) and
  C = A - pb2 (pure-bf16, 2x-eligible). PSUM fp32 reads cannot use the
  DVE 2x packed mode, so minimizing PSUM-touching ops is what matters.
- Output tiles are packed [120, 4*480] (folded) / [120, 2*480] (unfolded)
  per dma_start so each partition contributes one contiguous 3840B/1920B
  run: HWDGE descriptor generation costs ~7ns/descriptor on the issuing
  sequencer and dominated both the post-compute tail and mid-kernel
  sequencer occupancy when stores went out as 16 x 120-descriptor tiles.
  The very last tile splits across both queues to overlap the final issue
  and HBM write receipt.
"""

import numpy as np
import ml_dtypes

import concourse.bass as bass  # noqa: F401  (engine types route via nc)
import concourse.mybir as mybir
import concourse.tile as tile
from concourse import bacc
from concourse.bass_utils import run_bass_kernel_spmd

# Problem shape (hardcoded per contract)
B, C, H = 16, 3, 768
OUT = 960  # H * 1.25
N_CORES = 8
SLICES = (B * C) // N_CORES  # 6 per core

P = 128
HF = H // 2     # 384: folded input length
QF = OUT // 2   # 480: folded output length
H2 = HF // 2    # 192: doubly folded (level-2 contraction)
Q2F = QF // 2   # 240: doubly folded output rows
KT = HF // P    # 3 contraction tiles of 128
NT1 = HF // P   # 3 stage-1 output-row tiles
M2 = 120
MT2 = QF // M2  # 4 stage-2 output-row tiles

DT = mybir.dt.bfloat16
BF16 = ml_dtypes.bfloat16


def _build_consts():
    """Returns (mt, mt2a, mt2b) host arrays (bf16).

    Stage-1 runs in the j-folded output basis: columns jh<240 hold
    P[:, jh] + P[:, 479-jh], columns 240+jh hold the difference. For the
    MP-path products (t=0,2) the fold pairs with an m-fold of the input
    (S[m]=S[383-m], D[m]=-D[383-m]), halving the contraction to K=192.

    mt [128, 5*QF], stage-1 moving operands:
      slot 0: [S[0:128] (240 cols) | D[0:128] (240 cols)]   (MP-path kl0+/-)
      slot 1: merged kl1 [128, 480]: rows 0:64 = [S[128:192] | 0],
              rows 64:128 = [0 | D[128:192]]  (one full-row matmul covers
              both 64-row tails)
      slots 2-4: MMj = [MM^T col-sums | col-diffs] [384, 480], K-striped
              (MM-path, no m-fold; also the stage-2 lhsT for Q2/Q4).
    mt2a [128, 2*Q2F], mt2b [128, 2*Q2F]: level-2 constants MP2p^T/MP2m^T
         [192, 240] split into the K-chunks matching A's row tiling:
         f+ contracts A rows 0:192  = [kl0 (128) ; kl1[:64]  (64)]
         f- contracts A rows 192:384 = [kl1[64:] (64) ; kl2 (128)]
         mt2a[:, 0] = MP2pT[0:128]   mt2b[:, 0] = MP2pT[128:192]
         mt2b[:, 1] = MP2mT[0:64]    mt2a[:, 1] = MP2mT[64:192]
    """
    n = np.arange(H, dtype=np.float64)
    k = np.arange(H, dtype=np.float64)[:, None]
    D = 2.0 * np.cos(np.pi * (2.0 * n[None, :] + 1.0) * k / (2.0 * H))

    n2 = np.arange(OUT, dtype=np.float64)[:, None]
    k2 = np.arange(OUT, dtype=np.float64)[None, :]
    E = np.cos(np.pi * (2.0 * n2 + 1.0) * k2 / (2.0 * OUT)) / OUT
    E[:, 0] = 1.0 / (2.0 * OUT)

    M = E[:, :H] @ D  # [960, 768]
    MP = (M[:QF, :HF] + M[:QF, H - 1 : HF - 1 : -1]) / 2.0  # [480, 384]
    MM = (M[:QF, :HF] - M[:QF, H - 1 : HF - 1 : -1]) / 2.0

    MPT, MMT = MP.T, MM.T  # [384, 480]
    S = MPT[:, :Q2F] + MPT[:, QF - 1 : Q2F - 1 : -1]  # [384, 240], S[m]=S[383-m]
    Dm = MPT[:, :Q2F] - MPT[:, QF - 1 : Q2F - 1 : -1]  # D[m]=-D[383-m]
    MMj = np.concatenate(
        [MMT[:, :Q2F] + MMT[:, QF - 1 : Q2F - 1 : -1],
         MMT[:, :Q2F] - MMT[:, QF - 1 : Q2F - 1 : -1]], axis=1)  # [384, 480]

    mt = np.zeros((P, 5, QF), dtype=np.float64)
    mt[:, 0, :Q2F] = S[:P]
    mt[:, 0, Q2F:] = Dm[:P]
    mt[:64, 1, :Q2F] = S[P : P + 64]
    mt[64:, 1, Q2F:] = Dm[P : P + 64]
    mt[:, 2:5] = MMj.reshape(KT, P, QF).transpose(1, 0, 2)
    mt = np.ascontiguousarray(mt.reshape(P, 5 * QF)).astype(BF16)

    MP2pT = ((MP[:Q2F, :H2] + MP[:Q2F, HF - 1 : H2 - 1 : -1]) / 2.0).T  # [192, 240]
    MP2mT = ((MP[:Q2F, :H2] - MP[:Q2F, HF - 1 : H2 - 1 : -1]) / 2.0).T
    mt2a = np.stack([MP2pT[0:128], MP2mT[64:192]], axis=1)  # [128, 2, 240]
    # 64-row chunks parked at the base partition their rhs slice uses
    # (matmul requires lhsT/rhs base partitions to match):
    # [:64, 0] = MP2pT[128:192] (rhs base 0), [64:, 1] = MP2mT[0:64] (base 64)
    mt2b = np.zeros((P, 2, Q2F), dtype=np.float64)
    mt2b[:64, 0] = MP2pT[128:192]
    mt2b[64:, 1] = MP2mT[0:64]
    mt2a = np.ascontiguousarray(mt2a.reshape(P, 2 * Q2F)).astype(BF16)
    mt2b = np.ascontiguousarray(mt2b.reshape(P, 2 * Q2F)).astype(BF16)
    return mt, mt2a, mt2b


def _fold_inputs(x: np.ndarray) -> np.ndarray:
    """Host fold + lhsT striping: [B*C, 128, 4*KT*HF] bf16.

    Product order t=0..3 pairs (x_pp, MP-path), (x_pm, MM-path),
    (x_mp, MP-path), (x_mm, MM-path). x_pp and x_pm (the A/C path) are
    additionally row-permuted into level-2 folded order.

    MM-path lhsT chunks (t=1,3): x_sb[p, t, kl, n] = x_t[n, kl*128+p].
    MP-path chunks (t=0,2) are m-folded: with x2+ = x_t[:, m] + x_t[:, 383-m]
    and x2- the difference (m < 192):
      kl=0: x2+[n, p]          kl=1: x2-[n, p]
      kl=2 rows 0:64 = x2+[n, 128+p], rows 64:128 = x2-[n, 64+p]
    matching mt slots 0 (kl0+/kl0-) and 1 (merged kl1 tails).
    """
    xr = x.reshape(B * C, H, H)
    fp = xr[:, :, :HF] + xr[:, :, H - 1 : HF - 1 : -1]  # col fold
    fm = xr[:, :, :HF] - xr[:, :, H - 1 : HF - 1 : -1]
    xpp = fp[:, :HF] + fp[:, H - 1 : HF - 1 : -1]  # row fold
    xmp = fp[:, :HF] - fp[:, H - 1 : HF - 1 : -1]
    xpm = fm[:, :HF] + fm[:, H - 1 : HF - 1 : -1]
    xmm = fm[:, :HF] - fm[:, H - 1 : HF - 1 : -1]
    # level-2 row fold (permute) on the A/C path
    xpp = np.concatenate(
        [xpp[:, :H2] + xpp[:, HF - 1 : H2 - 1 : -1],
         xpp[:, :H2] - xpp[:, HF - 1 : H2 - 1 : -1]], axis=1)
    xpm = np.concatenate(
        [xpm[:, :H2] + xpm[:, HF - 1 : H2 - 1 : -1],
         xpm[:, :H2] - xpm[:, HF - 1 : H2 - 1 : -1]], axis=1)

    def mp_chunks(xt):
        # xt [s, n, m] -> m-folded lhsT chunks [s, KT, P, HF]
        xp2 = xt[:, :, :H2] + xt[:, :, HF - 1 : H2 - 1 : -1]  # [s, n, 192]
        xm2 = xt[:, :, :H2] - xt[:, :, HF - 1 : H2 - 1 : -1]
        out = np.empty((xt.shape[0], KT, P, HF), dtype=xt.dtype)
        out[:, 0] = xp2[:, :, :P].transpose(0, 2, 1)
        out[:, 1] = xm2[:, :, :P].transpose(0, 2, 1)
        out[:, 2, :64] = xp2[:, :, P:H2].transpose(0, 2, 1)
        out[:, 2, 64:] = xm2[:, :, P:H2].transpose(0, 2, 1)
        return out

    def mm_chunks(xt):
        return xt.transpose(0, 2, 1).reshape(xt.shape[0], KT, P, HF)

    xt_all = np.stack(
        [mp_chunks(xpp), mm_chunks(xpm), mp_chunks(xmp), mm_chunks(xmm)],
        axis=1)  # [s, 4, KT, P, HF]
    xt_all = xt_all.transpose(0, 3, 1, 2, 4)  # [s, P, 4, KT, HF]
    return np.ascontiguousarray(
        xt_all.reshape(B * C, P, 4 * KT * HF)).astype(BF16)


def _build_program():
    nc = bacc.Bacc(None, target_bir_lowering=False, debug=False)

    x_ext = nc.dram_tensor("x", [SLICES, P, 4 * KT * HF], DT, kind="ExternalInput")
    mt_ext = nc.dram_tensor("mt", [P, 5 * QF], DT, kind="ExternalInput")
    mt2a_ext = nc.dram_tensor("mt2a", [P, 2 * Q2F], DT, kind="ExternalInput")
    mt2b_ext = nc.dram_tensor("mt2b", [P, 2 * Q2F], DT, kind="ExternalInput")
    # Output tiles are packed 4-wide (folded) / 2-wide (unfolded) so each
    # partition contributes one 3840B/1920B contiguous run per dma_start:
    # HWDGE descriptor generation costs ~7ns/descriptor on the issuing
    # sequencer, and 16 x 120-descriptor stores per slice (~13us of issue)
    # dominated the post-compute tail. Host unscrambles the packing.
    outf_ext = nc.dram_tensor("outf", [SLICES, 2, M2, 4 * QF], DT, kind="ExternalOutput")
    outu_ext = nc.dram_tensor("outu", [SLICES, MT2, M2, 2 * QF], DT, kind="ExternalOutput")

    with tile.TileContext(nc) as tc:
        with (
            tc.tile_pool(name="const", bufs=1) as const_pool,
            tc.tile_pool(name="xp", bufs=4) as x_pool,
            tc.tile_pool(name="rp", bufs=2) as r_pool,
            tc.tile_pool(name="op", bufs=10) as o_pool,
            tc.tile_pool(name="ps", bufs=8, space="PSUM") as psum_pool,
        ):
            x_dram = x_ext[:].rearrange("s p (t k n) -> s p t k n", t=4, k=KT)

            # Slice-0 x is split per product t and mt per path so the first
            # stage-1 group's operands (MP-path consts + x t=0) land early
            # instead of waiting for the whole 1.9MB batch. scalar/sync HWDGE
            # queues drain round-robin, so the orders stay aligned with
            # consumption order. Later slices prefetch on sync with slack.
            mt_dram = mt_ext[:].rearrange("p (c j) -> p c j", c=5)
            mt_sb = const_pool.tile([P, 5, QF], DT, name="mt")
            x_first = x_pool.tile([P, 4, KT, HF], DT, tag="x")
            nc.scalar.dma_start(mt_sb[:, 0:2], mt_dram[:, 0:2])
            nc.sync.dma_start(x_first[:, 0], x_dram[0][:, 0])
            nc.scalar.dma_start(mt_sb[:, 2:5], mt_dram[:, 2:5])
            nc.sync.dma_start(x_first[:, 1], x_dram[0][:, 1])
            nc.sync.dma_start(x_first[:, 2], x_dram[0][:, 2])
            nc.sync.dma_start(x_first[:, 3], x_dram[0][:, 3])
            mt2a_sb = const_pool.tile([P, 2, Q2F], DT, name="mt2a")
            nc.scalar.dma_start(mt2a_sb[:], mt2a_ext[:].rearrange("p (c j) -> p c j", c=2))
            mt2b_sb = const_pool.tile([P, 2, Q2F], DT, name="mt2b")
            nc.scalar.dma_start(mt2b_sb[:], mt2b_ext[:].rearrange("p (c j) -> p c j", c=2))


            # PE warmup: dummy matmuls keep the tensor engine busy while the
            # first loads land, so the HAM clock gate is already at 2.4 GHz.
            warm_m = const_pool.tile([P, QF], DT, name="warm_m")
            nc.vector.memset(warm_m[:], 0.0)
            warm_ps = psum_pool.tile([P, QF], mybir.dt.float32, tag="ps", name="warm_ps")
            for _ in range(8):
                nc.tensor.matmul(warm_ps[:], warm_m[:, :P], warm_m[:], start=True, stop=True)

            for s in range(SLICES):
                if s == 0:
                    x_sb = x_first
                else:
                    x_sb = x_pool.tile([P, 4, KT, HF], DT, tag="x")
                    nc.sync.dma_start(x_sb[:], x_dram[s])

                # Stage 1 (j-folded basis): P_t computed per (pair, nt) into
                # ps_a (MP-path, m-folded: two N=240 matmuls + one merged
                # full-row N=480) and ps_b (MM-path, 3 full matmuls), then
                # butterflied into r_sb[:, u]: u=0:A, u=1:C (level-2-row-
                # folded), u=2:B, u=3:D; r_sb[p, u, nt, j] = U[nt*128+p, j].
                # Drain: ScalarE casts ps_b -> bf16, DVE adds A = ps_a + pb
                # (the only PSUM-touching ops), GpSimd computes C = A - 2*pb
                # from SBUF. For slice 0 the a/b groups are batched so the
                # first matmuls need only the small MP-path constants + x t=0.
                r_sb = r_pool.tile([P, 4, KT, QF], DT, tag="r")

                def a_group(pair, nt):
                    # start=True clears the whole PSUM bank (first_mm), so
                    # only the first matmul carries it; the kl0- matmul's
                    # columns then have has_written=0 -> overwrite semantics.
                    t = 2 * pair
                    ps_a = psum_pool.tile([P, QF], mybir.dt.float32, tag="ps", name="ps_a")
                    xc = x_sb[:, t, :, nt * P : (nt + 1) * P]
                    nc.tensor.matmul(ps_a[:, :Q2F], xc[:, 0], mt_sb[:, 0, :Q2F],
                                     start=True, stop=False, skip_group_check=True)
                    nc.tensor.matmul(ps_a[:, Q2F:], xc[:, 1], mt_sb[:, 0, Q2F:],
                                     start=False, stop=False, skip_group_check=True)
                    nc.tensor.matmul(ps_a[:], xc[:, 2], mt_sb[:, 1, :],
                                     start=False, stop=True, skip_group_check=True)
                    return ps_a

                def b_group(pair, nt):
                    t = 2 * pair + 1
                    ps_b = psum_pool.tile([P, QF], mybir.dt.float32, tag="ps", name="ps_b")
                    for kl in range(KT):
                        nc.tensor.matmul(
                            ps_b[:],
                            x_sb[:, t, kl, nt * P : (nt + 1) * P],
                            mt_sb[:, 2 + kl, :],
                            start=(kl == 0),
                            stop=(kl == KT - 1),
                        )
                    return ps_b

                def bfly(pair, nt, ps_a, ps_b):
                    # ScalarE emits pb2 = 2*P2 (bf16); A = 0.5*pb2 + ps_a on
                    # DVE (PSUM read, 1x); C = A - pb2 is then a pure-bf16
                    # tensor_tensor, eligible for the DVE 2x packed mode.
                    # GpSimd was tried for C and is ~2.5x slower per op, which
                    # starved stage 2 of the C/D planes and stalled the PE.
                    pb2 = o_pool.tile([P, QF], DT, tag="pb", bufs=3, name="pb2")
                    nc.scalar.activation(
                        pb2[:], ps_b[:], mybir.ActivationFunctionType.Copy,
                        scale=2.0,
                    )
                    nc.vector.scalar_tensor_tensor(
                        out=r_sb[:, 2 * pair, nt, :],
                        in0=pb2[:],
                        scalar=0.5,
                        in1=ps_a[:],
                        op0=mybir.AluOpType.mult,
                        op1=mybir.AluOpType.add,
                    )
                    nc.vector.tensor_sub(
                        out=r_sb[:, 2 * pair + 1, nt, :],
                        in0=r_sb[:, 2 * pair, nt, :],
                        in1=pb2[:],
                    )

                for pair in range(2):
                    if s == 0 and pair == 0:
                        # ramp: all a-groups need only the small MP-path
                        # constants + x t=0; b-groups then ride the next
                        # chunks while pair-1 data still streams in.
                        pas = [a_group(pair, nt) for nt in range(NT1)]
                        pbs = [b_group(pair, nt) for nt in range(NT1)]
                        for nt in range(NT1):
                            bfly(pair, nt, pas[nt], pbs[nt])
                    else:
                        for nt in range(NT1):
                            ps_a = a_group(pair, nt)
                            ps_b = b_group(pair, nt)
                            bfly(pair, nt, ps_a, ps_b)

                # Stage 2, folded products: Q1 = MP A (u=0 -> qi 0), Q3 = MP C
                # (u=1 -> qi 2) via the level-2 fold. Output rows = [f+ (240);
                # f- (240)], host unfolds. Per (u, block i) the f+ and f- out
                # tiles are built together: each gets its full-K=128 chunk
                # (kl0 / kl2, background-pipelined LDWEIGHTS), then the two
                # K=64 kl1 chunks issue back-to-back at base partitions 0 / 64
                # -> disjoint row-groups, so the PE streams them concurrently
                # (one 480-cycle slot instead of two).
                for i in range(2):
                    msl = slice(i * M2, (i + 1) * M2)
                    pss = {}
                    for u in (0, 1):
                        psf = psum_pool.tile([P, QF], mybir.dt.float32, tag="ps", name="psf")
                        psg = psum_pool.tile([P, QF], mybir.dt.float32, tag="ps", name="psg")
                        nc.tensor.matmul(
                            psf[:M2, :], mt2a_sb[:, 0, msl], r_sb[:, u, 0, :],
                            start=True, stop=False,
                        )
                        nc.tensor.matmul(
                            psg[:M2, :], mt2a_sb[:, 1, msl], r_sb[:, u, 2, :],
                            start=True, stop=False,
                        )
                        pss[u] = (psf, psg)
                    # K=64 kl1 chunks clustered: the first +/- pair pays one
                    # foreground weight load; the second reuses it. Within a
                    # pair the two matmuls sit in disjoint row-groups (base
                    # partitions 0 / 64) and stream concurrently.
                    for u in (0, 1):
                        psf, psg = pss[u]
                        nc.tensor.matmul(
                            psf[:M2, :], mt2b_sb[:64, 0, msl], r_sb[0:64, u, 1, :],
                            start=False, stop=True,
                        )
                        nc.tensor.matmul(
                            psg[:M2, :], mt2b_sb[64:, 1, msl], r_sb[64:P, u, 1, :],
                            start=False, stop=True,
                        )
                    oc4 = o_pool.tile([M2, 4, QF], DT, tag="o4", bufs=4, name="oc4")
                    for u in (0, 1):
                        psf, psg = pss[u]
                        nc.scalar.copy(oc4[:, 2 * u], psf[:M2, :])
                        nc.vector.tensor_copy(oc4[:, 2 * u + 1], psg[:M2, :])
                    eng = nc.scalar if i == 0 else nc.sync
                    eng.dma_start(
                        outf_ext[s, i].rearrange("m (c j) -> m c j", c=4), oc4[:]
                    )

                # Stage 2, unfolded pair: Q2 = MM B (qi 1), Q4 = MM D (qi 3),
                # 12 matmuls each, computed in row-folded form: the lhsT MMj
                # (mt slots 2-4) is exactly (MMrow+ | MMrow-)^T, so output
                # rows are [Q2j+ (240); Q2j- (240)] and the host row-unfolds.
                # Q2/Q4 share stationary weights: adjacent matmuls with
                # identical lhsT keep the PE's weight-load pipelined. Drains
                # are pure casts, alternating ScalarE / DVE. Host does the
                # +- butterfly.
                for mi in range(MT2):
                    ps1 = psum_pool.tile([P, QF], mybir.dt.float32, tag="ps", name="ps1")
                    ps2 = psum_pool.tile([P, QF], mybir.dt.float32, tag="ps", name="ps2")
                    po1, po2 = ps1[:M2, :], ps2[:M2, :]
                    for kl in range(KT):
                        lh = mt_sb[:, 2 + kl, mi * M2 : (mi + 1) * M2]
                        nc.tensor.matmul(
                            po1, lh, r_sb[:, 2, kl, :],
                            start=(kl == 0), stop=(kl == KT - 1),
                        )
                        nc.tensor.matmul(
                            po2, lh, r_sb[:, 3, kl, :],
                            start=(kl == 0), stop=(kl == KT - 1),
                        )
                    oc2 = o_pool.tile([M2, 2, QF], DT, tag="o2", bufs=6, name="oc2")
                    nc.scalar.copy(oc2[:, 0], po1)
                    nc.vector.tensor_copy(oc2[:, 1], po2)
                    ou = outu_ext[s, mi].rearrange("m (c j) -> m c j", c=2)
                    if s == SLICES - 1 and mi == MT2 - 1:
                        # last tile of the kernel: split across both queues so
                        # the final issue + HBM write-receipt overlap.
                        nc.scalar.dma_start(ou[:, 0], oc2[:, 0])
                        nc.sync.dma_start(ou[:, 1], oc2[:, 1])
                    else:
                        eng = nc.sync if mi % 2 == 0 else nc.scalar
                        eng.dma_start(ou, oc2[:])

    nc.compile()
    return nc


_CACHE: dict = {}


def _get_program():
    if "nc" not in _CACHE:
        _CACHE["nc"] = _build_program()
        _CACHE["consts"] = _build_consts()
    return _CACHE["nc"], _CACHE["consts"]


def kernel(x: np.ndarray, _trace: bool = False):
    assert x.shape == (B, C, H, H), x.shape
    nc, (mt, mt2a, mt2b) = _get_program()
    x = np.ascontiguousarray(x, dtype=np.float32)
    x_arr = _fold_inputs(x)
    per_core = B // N_CORES
    in_maps = [
        {"x": x_arr[i * SLICES : (i + 1) * SLICES], "mt": mt, "mt2a": mt2a,
         "mt2b": mt2b}
        for i in range(N_CORES)
    ]
    res = run_bass_kernel_spmd(nc, in_maps, list(range(N_CORES)), trace=_trace)

    def row_unfold(qu, half):
        # rows [sum(240); diff(240)] -> natural 480 rows (x 0.5 on host for
        # the MMj fold; the level-2 constants have the 0.5 built in)
        return np.concatenate(
            [qu[..., :Q2F, :] + qu[..., Q2F:, :],
             (qu[..., :Q2F, :] - qu[..., Q2F:, :])[..., ::-1, :]],
            axis=-2) * half

    def col_unfold(qu):
        # cols [sum(240) | diff(240)] -> natural 480 cols (j-fold, x 0.5)
        return np.concatenate(
            [qu[..., :Q2F] + qu[..., Q2F:],
             (qu[..., :Q2F] - qu[..., Q2F:])[..., ::-1]],
            axis=-1) * 0.5

    out = np.empty((B, C, OUT, OUT), dtype=np.float32)
    for i in range(N_CORES):
        # outf [s, i2, m, 4, QF]: slots (0,1) = Q1j (f+ block i2, f- block
        # i2), (2,3) = Q3j. outu [s, mi, m, 2, QF]: slots = (Q2j, Q4j) rows
        # mi*120. Rebuild the [480, 480] folded-basis products.
        qf = np.asarray(res.results[i]["outf"]).astype(np.float32)
        qf = qf.reshape(per_core, C, 2, M2, 4, QF)
        qu = np.asarray(res.results[i]["outu"]).astype(np.float32)
        qu = qu.reshape(per_core, C, MT2, M2, 2, QF)
        q1j = qf[:, :, :, :, (0, 1)].transpose(0, 1, 4, 2, 3, 5).reshape(
            per_core, C, QF, QF)
        q3j = qf[:, :, :, :, (2, 3)].transpose(0, 1, 4, 2, 3, 5).reshape(
            per_core, C, QF, QF)
        q2j = qu[:, :, :, :, 0].reshape(per_core, C, QF, QF)
        q4j = qu[:, :, :, :, 1].reshape(per_core, C, QF, QF)
        q1 = col_unfold(row_unfold(q1j, 1.0))
        q2 = col_unfold(row_unfold(q2j, 0.5))
        q3 = col_unfold(row_unfold(q3j, 1.0))
        q4 = col_unfold(row_unfold(q4j, 0.5))
        blk = out[i * per_core : (i + 1) * per_core]
        blk[:, :, :QF, :QF] = q1 + q2
        blk[:, :, QF:, :QF] = (q1 - q2)[:, :, ::-1, :]
        blk[:, :, :QF, QF:] = (q3 + q4)[:, :, :, ::-1]
        blk[:, :, QF:, QF:] = (q3 - q4)[:, :, ::-1, ::-1]
    if _trace:
        return out, res
    return out



# revision 31
# speedup vs baseline: 1.0058x; 1.0058x over previous
"""DCT-based 1.25x upsample (2D DCT-II -> zero-pad spectrum -> 2D IDCT).

The reference computation is linear per (b, c) slice: out = M @ x @ M^T with
M = E960[:, :768] @ D768 (960x768). M is *centrosymmetric*
(M[959-i, 767-n] = M[i, n]), so the symmetric/antisymmetric fold halves the
matmul FLOPs:

    MP = (M[:480, :384] + M[:480, 767:383:-1]) / 2     [480, 384]
    MM = (M[:480, :384] - M[:480, 767:383:-1]) / 2
    x_pq = (row-fold p)(col-fold q)(x)                 4 tiles of [384, 384]

    P1 = x_pp MP^T   P2 = x_pm MM^T   P3 = x_mp MP^T   P4 = x_mm MM^T
    A = P1+P2  C = P1-P2  B = P3+P4  D = P3-P4         [384, 480] each
    Q1 = MP A   Q2 = MM B   Q3 = MP C   Q4 = MM D      [480, 480] each
    out quadrants = (Q1 +- Q2, Q3 +- Q4) with flips    (done on HOST)

Three further folds, all with the O(N^2) pre/post work on the host:

1. Level-2 row fold (MP centrosymmetric): x_pp/x_pm are row-permuted into
   folded order so A/C come out of stage 1 row-folded, and Q1/Q3 contract
   K=192 per output half (mt2a/mt2b constants). The two 64-row kl1 tails
   are issued back-to-back at base partitions 0/64 -> disjoint PE
   row-groups, streaming concurrently (one 480-cycle slot for both).
2. j-fold (stage-1 output basis): all stage-1 products are computed in the
   column basis [P(:,jh)+P(:,479-jh) | difference]. For the MP-path
   products the fold pairs with an m-fold of the input (S[m]=S[383-m]),
   halving stage-1 t=0,2 contraction to K=192: two N=240 matmuls plus one
   merged full-row matmul whose constant rhs is block-diagonal over the
   two 64-row tails. The j columns are inert through stage 2; the host
   unfolds them at the end.
3. Row fold on Q2/Q4 for free: the j-folded MM-path weights MMj equal
   (MMrow+ | MMrow-)^T, so the same mt slots serve as stage-2 lhsT and
   Q2/Q4 come out row-folded (host unfolds).

Per-slice PE work: stage 1 ~6.1us, folded stage 2 ~2.9us, unfolded ~5.0us
(~92% of pure streaming cycles; LDWEIGHTS pipelines in the background for
full-row weights, only partial-row loads serialize ~107ns).

Schedule notes (measured on trn2):
- The HAM clock gate runs the PE at half speed until ~3.4us of sustained
  activity; any PE idle gap risks re-throttle. Warmup matmuls bridge until
  the first x chunk + MP-path constants land (~12us incl. the ~2us HBM
  write-receipt latency on DMA completion semaphores); slice-0 stage-1
  runs its three a-groups first since they need only mt slots 0:2 + x t0.
- Slice-0 x is split per product t on the sync HWDGE queue; constants are
  split MP-path-first on the scalar queue (queues drain round-robin, so
  order on each queue matches consumption). The gpsimd queue is avoided:
  SWDGE starts slow and costs a ~5us DGE drain at teardown. GpSimd compute
  is also ~2.5x slower per elementwise op than DVE - keep it idle.
- Stage-1 drain per step: ScalarE casts pb2=2*P2 (bf16), DVE computes
  A = 0.5*pb2 + ps_a (the only PSUM reads, stuck at 1x mode) and
  C = A - pb2 (pure-bf16, 2x-eligible). PSUM fp32 reads cannot use the
  DVE 2x packed mode, so minimizing PSUM-touching ops is what matters.
- Output tiles are packed [120, 4*480] (folded) / [120, 2*480] (unfolded)
  per dma_start so each partition contributes one contiguous 3840B/1920B
  run: HWDGE descriptor generation costs ~7ns/descriptor on the issuing
  sequencer and dominated both the post-compute tail and mid-kernel
  sequencer occupancy when stores went out as 16 x 120-descriptor tiles.
  The very last tile splits across both queues to overlap the final issue
  and HBM write receipt.
"""

import numpy as np
import ml_dtypes

import concourse.bass as bass  # noqa: F401  (engine types route via nc)
import concourse.mybir as mybir
import concourse.tile as tile
from concourse import bacc
from concourse.bass_utils import run_bass_kernel_spmd

# Problem shape (hardcoded per contract)
B, C, H = 16, 3, 768
OUT = 960  # H * 1.25
N_CORES = 8
SLICES = (B * C) // N_CORES  # 6 per core

P = 128
HF = H // 2     # 384: folded input length
QF = OUT // 2   # 480: folded output length
H2 = HF // 2    # 192: doubly folded (level-2 contraction)
Q2F = QF // 2   # 240: doubly folded output rows
KT = HF // P    # 3 contraction tiles of 128
NT1 = HF // P   # 3 stage-1 output-row tiles
M2 = 120
MT2 = QF // M2  # 4 stage-2 output-row tiles

DT = mybir.dt.bfloat16
BF16 = ml_dtypes.bfloat16


def _build_consts():
    """Returns (mt, mt2a, mt2b) host arrays (bf16).

    Stage-1 runs in the j-folded output basis: columns jh<240 hold
    P[:, jh] + P[:, 479-jh], columns 240+jh hold the difference. For the
    MP-path products (t=0,2) the fold pairs with an m-fold of the input
    (S[m]=S[383-m], D[m]=-D[383-m]), halving the contraction to K=192.

    mt [128, 5*QF], stage-1 moving operands:
      slot 0: [S[0:128] (240 cols) | D[0:128] (240 cols)]   (MP-path kl0+/-)
      slot 1: merged kl1 [128, 480]: rows 0:64 = [S[128:192] | 0],
              rows 64:128 = [0 | D[128:192]]  (one full-row matmul covers
              both 64-row tails)
      slots 2-4: MMj = [MM^T col-sums | col-diffs] [384, 480], K-striped
              (MM-path, no m-fold; also the stage-2 lhsT for Q2/Q4).
    mt2a [128, 2*Q2F], mt2b [128, 2*Q2F]: level-2 constants MP2p^T/MP2m^T
         [192, 240] split into the K-chunks matching A's row tiling:
         f+ contracts A rows 0:192  = [kl0 (128) ; kl1[:64]  (64)]
         f- contracts A rows 192:384 = [kl1[64:] (64) ; kl2 (128)]
         mt2a[:, 0] = MP2pT[0:128]   mt2b[:, 0] = MP2pT[128:192]
         mt2b[:, 1] = MP2mT[0:64]    mt2a[:, 1] = MP2mT[64:192]
    """
    n = np.arange(H, dtype=np.float64)
    k = np.arange(H, dtype=np.float64)[:, None]
    D = 2.0 * np.cos(np.pi * (2.0 * n[None, :] + 1.0) * k / (2.0 * H))

    n2 = np.arange(OUT, dtype=np.float64)[:, None]
    k2 = np.arange(OUT, dtype=np.float64)[None, :]
    E = np.cos(np.pi * (2.0 * n2 + 1.0) * k2 / (2.0 * OUT)) / OUT
    E[:, 0] = 1.0 / (2.0 * OUT)

    M = E[:, :H] @ D  # [960, 768]
    MP = (M[:QF, :HF] + M[:QF, H - 1 : HF - 1 : -1]) / 2.0  # [480, 384]
    MM = (M[:QF, :HF] - M[:QF, H - 1 : HF - 1 : -1]) / 2.0

    MPT, MMT = MP.T, MM.T  # [384, 480]
    S = MPT[:, :Q2F] + MPT[:, QF - 1 : Q2F - 1 : -1]  # [384, 240], S[m]=S[383-m]
    Dm = MPT[:, :Q2F] - MPT[:, QF - 1 : Q2F - 1 : -1]  # D[m]=-D[383-m]
    MMj = np.concatenate(
        [MMT[:, :Q2F] + MMT[:, QF - 1 : Q2F - 1 : -1],
         MMT[:, :Q2F] - MMT[:, QF - 1 : Q2F - 1 : -1]], axis=1)  # [384, 480]

    mt = np.zeros((P, 5, QF), dtype=np.float64)
    mt[:, 0, :Q2F] = S[:P]
    mt[:, 0, Q2F:] = Dm[:P]
    mt[:64, 1, :Q2F] = S[P : P + 64]
    mt[64:, 1, Q2F:] = Dm[P : P + 64]
    mt[:, 2:5] = MMj.reshape(KT, P, QF).transpose(1, 0, 2)
    mt = np.ascontiguousarray(mt.reshape(P, 5 * QF)).astype(BF16)

    MP2pT = ((MP[:Q2F, :H2] + MP[:Q2F, HF - 1 : H2 - 1 : -1]) / 2.0).T  # [192, 240]
    MP2mT = ((MP[:Q2F, :H2] - MP[:Q2F, HF - 1 : H2 - 1 : -1]) / 2.0).T
    mt2a = np.stack([MP2pT[0:128], MP2mT[64:192]], axis=1)  # [128, 2, 240]
    # 64-row chunks parked at the base partition their rhs slice uses
    # (matmul requires lhsT/rhs base partitions to match):
    # [:64, 0] = MP2pT[128:192] (rhs base 0), [64:, 1] = MP2mT[0:64] (base 64)
    mt2b = np.zeros((P, 2, Q2F), dtype=np.float64)
    mt2b[:64, 0] = MP2pT[128:192]
    mt2b[64:, 1] = MP2mT[0:64]
    mt2a = np.ascontiguousarray(mt2a.reshape(P, 2 * Q2F)).astype(BF16)
    mt2b = np.ascontiguousarray(mt2b.reshape(P, 2 * Q2F)).astype(BF16)
    return mt, mt2a, mt2b


def _fold_inputs(x: np.ndarray) -> np.ndarray:
    """Host fold + lhsT striping: [B*C, 128, 4*KT*HF] bf16.

    Product order t=0..3 pairs (x_pp, MP-path), (x_pm, MM-path),
    (x_mp, MP-path), (x_mm, MM-path). x_pp and x_pm (the A/C path) are
    additionally row-permuted into level-2 folded order.

    MM-path lhsT chunks (t=1,3): x_sb[p, t, kl, n] = x_t[n, kl*128+p].
    MP-path chunks (t=0,2) are m-folded: with x2+ = x_t[:, m] + x_t[:, 383-m]
    and x2- the difference (m < 192):
      kl=0: x2+[n, p]          kl=1: x2-[n, p]
      kl=2 rows 0:64 = x2+[n, 128+p], rows 64:128 = x2-[n, 64+p]
    matching mt slots 0 (kl0+/kl0-) and 1 (merged kl1 tails).
    """
    xr = x.reshape(B * C, H, H)
    fp = xr[:, :, :HF] + xr[:, :, H - 1 : HF - 1 : -1]  # col fold
    fm = xr[:, :, :HF] - xr[:, :, H - 1 : HF - 1 : -1]
    xpp = fp[:, :HF] + fp[:, H - 1 : HF - 1 : -1]  # row fold
    xmp = fp[:, :HF] - fp[:, H - 1 : HF - 1 : -1]
    xpm = fm[:, :HF] + fm[:, H - 1 : HF - 1 : -1]
    xmm = fm[:, :HF] - fm[:, H - 1 : HF - 1 : -1]
    # level-2 row fold (permute) on the A/C path
    xpp = np.concatenate(
        [xpp[:, :H2] + xpp[:, HF - 1 : H2 - 1 : -1],
         xpp[:, :H2] - xpp[:, HF - 1 : H2 - 1 : -1]], axis=1)
    xpm = np.concatenate(
        [xpm[:, :H2] + xpm[:, HF - 1 : H2 - 1 : -1],
         xpm[:, :H2] - xpm[:, HF - 1 : H2 - 1 : -1]], axis=1)

    def mp_chunks(xt):
        # xt [s, n, m] -> m-folded lhsT chunks [s, KT, P, HF]
        xp2 = xt[:, :, :H2] + xt[:, :, HF - 1 : H2 - 1 : -1]  # [s, n, 192]
        xm2 = xt[:, :, :H2] - xt[:, :, HF - 1 : H2 - 1 : -1]
        out = np.empty((xt.shape[0], KT, P, HF), dtype=xt.dtype)
        out[:, 0] = xp2[:, :, :P].transpose(0, 2, 1)
        out[:, 1] = xm2[:, :, :P].transpose(0, 2, 1)
        out[:, 2, :64] = xp2[:, :, P:H2].transpose(0, 2, 1)
        out[:, 2, 64:] = xm2[:, :, P:H2].transpose(0, 2, 1)
        return out

    def mm_chunks(xt):
        return xt.transpose(0, 2, 1).reshape(xt.shape[0], KT, P, HF)

    xt_all = np.stack(
        [mp_chunks(xpp), mm_chunks(xpm), mp_chunks(xmp), mm_chunks(xmm)],
        axis=1)  # [s, 4, KT, P, HF]
    xt_all = xt_all.transpose(0, 3, 1, 2, 4)  # [s, P, 4, KT, HF]
    return np.ascontiguousarray(
        xt_all.reshape(B * C, P, 4 * KT * HF)).astype(BF16)


def _build_program():
    nc = bacc.Bacc(None, target_bir_lowering=False, debug=False)

    x_ext = nc.dram_tensor("x", [SLICES, P, 4 * KT * HF], DT, kind="ExternalInput")
    mt_ext = nc.dram_tensor("mt", [P, 5 * QF], DT, kind="ExternalInput")
    mt2a_ext = nc.dram_tensor("mt2a", [P, 2 * Q2F], DT, kind="ExternalInput")
    mt2b_ext = nc.dram_tensor("mt2b", [P, 2 * Q2F], DT, kind="ExternalInput")
    # Output tiles are packed 4-wide (folded) / 2-wide (unfolded) so each
    # partition contributes one 3840B/1920B contiguous run per dma_start:
    # HWDGE descriptor generation costs ~7ns/descriptor on the issuing
    # sequencer, and 16 x 120-descriptor stores per slice (~13us of issue)
    # dominated the post-compute tail. Host unscrambles the packing.
    outf_ext = nc.dram_tensor("outf", [SLICES, 2, M2, 4 * QF], DT, kind="ExternalOutput")
    outu_ext = nc.dram_tensor("outu", [SLICES, MT2, M2, 2 * QF], DT, kind="ExternalOutput")

    with tile.TileContext(nc) as tc:
        with (
            tc.tile_pool(name="const", bufs=1) as const_pool,
            tc.tile_pool(name="xp", bufs=3) as x_pool,
            tc.tile_pool(name="rp", bufs=2) as r_pool,
            tc.tile_pool(name="op", bufs=10) as o_pool,
            tc.tile_pool(name="ps", bufs=8, space="PSUM") as psum_pool,
        ):
            x_dram = x_ext[:].rearrange("s p (t k n) -> s p t k n", t=4, k=KT)

            # Slice-0 x is split per product t and mt per path so the first
            # stage-1 group's operands (MP-path consts + x t=0) land early
            # instead of waiting for the whole 1.9MB batch. scalar/sync HWDGE
            # queues drain round-robin, so the orders stay aligned with
            # consumption order. Later slices prefetch on sync with slack.
            mt_dram = mt_ext[:].rearrange("p (c j) -> p c j", c=5)
            mt_sb = const_pool.tile([P, 5, QF], DT, name="mt")
            x_first = x_pool.tile([P, 4, KT, HF], DT, tag="x")
            nc.scalar.dma_start(mt_sb[:, 0:2], mt_dram[:, 0:2])
            nc.sync.dma_start(x_first[:, 0], x_dram[0][:, 0])
            nc.scalar.dma_start(mt_sb[:, 2:5], mt_dram[:, 2:5])
            nc.sync.dma_start(x_first[:, 1], x_dram[0][:, 1])
            nc.sync.dma_start(x_first[:, 2], x_dram[0][:, 2])
            nc.sync.dma_start(x_first[:, 3], x_dram[0][:, 3])
            mt2a_sb = const_pool.tile([P, 2, Q2F], DT, name="mt2a")
            nc.scalar.dma_start(mt2a_sb[:], mt2a_ext[:].rearrange("p (c j) -> p c j", c=2))
            mt2b_sb = const_pool.tile([P, 2, Q2F], DT, name="mt2b")
            nc.scalar.dma_start(mt2b_sb[:], mt2b_ext[:].rearrange("p (c j) -> p c j", c=2))


            # PE warmup: dummy matmuls keep the tensor engine busy while the
            # first loads land, so the HAM clock gate is already at 2.4 GHz.
            warm_m = const_pool.tile([P, QF], DT, name="warm_m")
            nc.vector.memset(warm_m[:], 0.0)
            warm_ps = psum_pool.tile([P, QF], mybir.dt.float32, tag="ps", name="warm_ps")
            for _ in range(8):
                nc.tensor.matmul(warm_ps[:], warm_m[:, :P], warm_m[:], start=True, stop=True)

            for s in range(SLICES):
                if s == 0:
                    x_sb = x_first
                else:
                    x_sb = x_pool.tile([P, 4, KT, HF], DT, tag="x")
                    nc.sync.dma_start(x_sb[:], x_dram[s])

                # Stage 1 (j-folded basis): P_t computed per (pair, nt) into
                # ps_a (MP-path, m-folded: two N=240 matmuls + one merged
                # full-row N=480) and ps_b (MM-path, 3 full matmuls), then
                # butterflied into r_sb[:, u]: u=0:A, u=1:C (level-2-row-
                # folded), u=2:B, u=3:D; r_sb[p, u, nt, j] = U[nt*128+p, j].
                # Drain: ScalarE casts ps_b -> bf16, DVE adds A = ps_a + pb
                # (the only PSUM-touching ops), GpSimd computes C = A - 2*pb
                # from SBUF. For slice 0 the a/b groups are batched so the
                # first matmuls need only the small MP-path constants + x t=0.
                r_sb = r_pool.tile([P, 4, KT, QF], DT, tag="r")

                def a_group(pair, nt):
                    # start=True clears the whole PSUM bank (first_mm), so
                    # only the first matmul carries it; the kl0- matmul's
                    # columns then have has_written=0 -> overwrite semantics.
                    t = 2 * pair
                    ps_a = psum_pool.tile([P, QF], mybir.dt.float32, tag="ps", name="ps_a")
                    xc = x_sb[:, t, :, nt * P : (nt + 1) * P]
                    nc.tensor.matmul(ps_a[:, :Q2F], xc[:, 0], mt_sb[:, 0, :Q2F],
                                     start=True, stop=False, skip_group_check=True)
                    nc.tensor.matmul(ps_a[:, Q2F:], xc[:, 1], mt_sb[:, 0, Q2F:],
                                     start=False, stop=False, skip_group_check=True)
                    nc.tensor.matmul(ps_a[:], xc[:, 2], mt_sb[:, 1, :],
                                     start=False, stop=True, skip_group_check=True)
                    return ps_a

                def b_group(pair, nt):
                    t = 2 * pair + 1
                    ps_b = psum_pool.tile([P, QF], mybir.dt.float32, tag="ps", name="ps_b")
                    for kl in range(KT):
                        nc.tensor.matmul(
                            ps_b[:],
                            x_sb[:, t, kl, nt * P : (nt + 1) * P],
                            mt_sb[:, 2 + kl, :],
                            start=(kl == 0),
                            stop=(kl == KT - 1),
                        )
                    return ps_b

                def bfly(pair, nt, ps_a, ps_b):
                    # ScalarE emits pb2 = 2*P2 (bf16); A = 0.5*pb2 + ps_a on
                    # DVE (PSUM read, 1x); C = A - pb2 is then a pure-bf16
                    # tensor_tensor, eligible for the DVE 2x packed mode.
                    # GpSimd was tried for C and is ~2.5x slower per op, which
                    # starved stage 2 of the C/D planes and stalled the PE.
                    pb2 = o_pool.tile([P, QF], DT, tag="pb", bufs=3, name="pb2")
                    nc.scalar.activation(
                        pb2[:], ps_b[:], mybir.ActivationFunctionType.Copy,
                        scale=2.0,
                    )
                    nc.vector.scalar_tensor_tensor(
                        out=r_sb[:, 2 * pair, nt, :],
                        in0=pb2[:],
                        scalar=0.5,
                        in1=ps_a[:],
                        op0=mybir.AluOpType.mult,
                        op1=mybir.AluOpType.add,
                    )
                    nc.vector.tensor_sub(
                        out=r_sb[:, 2 * pair + 1, nt, :],
                        in0=r_sb[:, 2 * pair, nt, :],
                        in1=pb2[:],
                    )

                for pair in range(2):
                    if s == 0 and pair == 0:
                        # ramp: all a-groups need only the small MP-path
                        # constants + x t=0; b-groups then ride the next
                        # chunks while pair-1 data still streams in.
                        pas = [a_group(pair, nt) for nt in range(NT1)]
                        pbs = [b_group(pair, nt) for nt in range(NT1)]
                        for nt in range(NT1):
                            bfly(pair, nt, pas[nt], pbs[nt])
                    else:
                        for nt in range(NT1):
                            ps_a = a_group(pair, nt)
                            ps_b = b_group(pair, nt)
                            bfly(pair, nt, ps_a, ps_b)

                # Stage 2, folded products: Q1 = MP A (u=0 -> qi 0), Q3 = MP C
                # (u=1 -> qi 2) via the level-2 fold. Output rows = [f+ (240);
                # f- (240)], host unfolds. Per (u, block i) the f+ and f- out
                # tiles are built together: each gets its full-K=128 chunk
                # (kl0 / kl2, background-pipelined LDWEIGHTS), then the two
                # K=64 kl1 chunks issue back-to-back at base partitions 0 / 64
                # -> disjoint row-groups, so the PE streams them concurrently
                # (one 480-cycle slot instead of two).
                for i in range(2):
                    msl = slice(i * M2, (i + 1) * M2)
                    pss = {}
                    for u in (0, 1):
                        psf = psum_pool.tile([P, QF], mybir.dt.float32, tag="ps", name="psf")
                        psg = psum_pool.tile([P, QF], mybir.dt.float32, tag="ps", name="psg")
                        nc.tensor.matmul(
                            psf[:M2, :], mt2a_sb[:, 0, msl], r_sb[:, u, 0, :],
                            start=True, stop=False,
                        )
                        nc.tensor.matmul(
                            psg[:M2, :], mt2a_sb[:, 1, msl], r_sb[:, u, 2, :],
                            start=True, stop=False,
                        )
                        pss[u] = (psf, psg)
                    # K=64 kl1 chunks clustered: the first +/- pair pays one
                    # foreground weight load; the second reuses it. Within a
                    # pair the two matmuls sit in disjoint row-groups (base
                    # partitions 0 / 64) and stream concurrently.
                    for u in (0, 1):
                        psf, psg = pss[u]
                        nc.tensor.matmul(
                            psf[:M2, :], mt2b_sb[:64, 0, msl], r_sb[0:64, u, 1, :],
                            start=False, stop=True,
                        )
                        nc.tensor.matmul(
                            psg[:M2, :], mt2b_sb[64:, 1, msl], r_sb[64:P, u, 1, :],
                            start=False, stop=True,
                        )
                    oc4 = o_pool.tile([M2, 4, QF], DT, tag="o4", bufs=3, name="oc4")
                    for u in (0, 1):
                        psf, psg = pss[u]
                        nc.scalar.copy(oc4[:, 2 * u], psf[:M2, :])
                        nc.vector.tensor_copy(oc4[:, 2 * u + 1], psg[:M2, :])
                    eng = nc.scalar if i == 0 else nc.sync
                    eng.dma_start(
                        outf_ext[s, i].rearrange("m (c j) -> m c j", c=4), oc4[:]
                    )

                # Stage 2, unfolded pair: Q2 = MM B (qi 1), Q4 = MM D (qi 3),
                # 12 matmuls each, computed in row-folded form: the lhsT MMj
                # (mt slots 2-4) is exactly (MMrow+ | MMrow-)^T, so output
                # rows are [Q2j+ (240); Q2j- (240)] and the host row-unfolds.
                # Q2/Q4 share stationary weights: adjacent matmuls with
                # identical lhsT keep the PE's weight-load pipelined. Drains
                # are pure casts, alternating ScalarE / DVE. Host does the
                # +- butterfly.
                for mi in range(MT2):
                    ps1 = psum_pool.tile([P, QF], mybir.dt.float32, tag="ps", name="ps1")
                    ps2 = psum_pool.tile([P, QF], mybir.dt.float32, tag="ps", name="ps2")
                    po1, po2 = ps1[:M2, :], ps2[:M2, :]
                    for kl in range(KT):
                        lh = mt_sb[:, 2 + kl, mi * M2 : (mi + 1) * M2]
                        nc.tensor.matmul(
                            po1, lh, r_sb[:, 2, kl, :],
                            start=(kl == 0), stop=(kl == KT - 1),
                        )
                        nc.tensor.matmul(
                            po2, lh, r_sb[:, 3, kl, :],
                            start=(kl == 0), stop=(kl == KT - 1),
                        )
                    oc2 = o_pool.tile([M2, 2, QF], DT, tag="o2", bufs=5, name="oc2")
                    nc.scalar.copy(oc2[:, 0], po1)
                    nc.vector.tensor_copy(oc2[:, 1], po2)
                    ou = outu_ext[s, mi].rearrange("m (c j) -> m c j", c=2)
                    if s == SLICES - 1 and mi == MT2 - 1:
                        # last tile of the kernel: split across both queues so
                        # the final issue + HBM write-receipt overlap.
                        nc.scalar.dma_start(ou[:, 0], oc2[:, 0])
                        nc.sync.dma_start(ou[:, 1], oc2[:, 1])
                    else:
                        eng = nc.sync if mi % 2 == 0 else nc.scalar
                        eng.dma_start(ou, oc2[:])

    nc.compile()
    return nc


_CACHE: dict = {}


def _get_program():
    if "nc" not in _CACHE:
        _CACHE["nc"] = _build_program()
        _CACHE["consts"] = _build_consts()
    return _CACHE["nc"], _CACHE["consts"]


def kernel(x: np.ndarray, _trace: bool = False):
    assert x.shape == (B, C, H, H), x.shape
    nc, (mt, mt2a, mt2b) = _get_program()
    x = np.ascontiguousarray(x, dtype=np.float32)
    x_arr = _fold_inputs(x)
    per_core = B // N_CORES
    in_maps = [
        {"x": x_arr[i * SLICES : (i + 1) * SLICES], "mt": mt, "mt2a": mt2a,
         "mt2b": mt2b}
        for i in range(N_CORES)
    ]
    res = run_bass_kernel_spmd(nc, in_maps, list(range(N_CORES)), trace=_trace)

    def row_unfold(qu, half):
        # rows [sum(240); diff(240)] -> natural 480 rows (x 0.5 on host for
        # the MMj fold; the level-2 constants have the 0.5 built in)
        return np.concatenate(
            [qu[..., :Q2F, :] + qu[..., Q2F:, :],
             (qu[..., :Q2F, :] - qu[..., Q2F:, :])[..., ::-1, :]],
            axis=-2) * half

    def col_unfold(qu):
        # cols [sum(240) | diff(240)] -> natural 480 cols (j-fold, x 0.5)
        return np.concatenate(
            [qu[..., :Q2F] + qu[..., Q2F:],
             (qu[..., :Q2F] - qu[..., Q2F:])[..., ::-1]],
            axis=-1) * 0.5

    out = np.empty((B, C, OUT, OUT), dtype=np.float32)
    for i in range(N_CORES):
        # outf [s, i2, m, 4, QF]: slots (0,1) = Q1j (f+ block i2, f- block
        # i2), (2,3) = Q3j. outu [s, mi, m, 2, QF]: slots = (Q2j, Q4j) rows
        # mi*120. Rebuild the [480, 480] folded-basis products.
        qf = np.asarray(res.results[i]["outf"]).astype(np.float32)
        qf = qf.reshape(per_core, C, 2, M2, 4, QF)
        qu = np.asarray(res.results[i]["outu"]).astype(np.float32)
        qu = qu.reshape(per_core, C, MT2, M2, 2, QF)
        q1j = qf[:, :, :, :, (0, 1)].transpose(0, 1, 4, 2, 3, 5).reshape(
            per_core, C, QF, QF)
        q3j = qf[:, :, :, :, (2, 3)].transpose(0, 1, 4, 2, 3, 5).reshape(
            per_core, C, QF, QF)
        q2j = qu[:, :, :, :, 0].reshape(per_core, C, QF, QF)
        q4j = qu[:, :, :, :, 1].reshape(per_core, C, QF, QF)
        q1 = col_unfold(row_unfold(q1j, 1.0))
        q2 = col_unfold(row_unfold(q2j, 0.5))
        q3 = col_unfold(row_unfold(q3j, 1.0))
        q4 = col_unfold(row_unfold(q4j, 0.5))
        blk = out[i * per_core : (i + 1) * per_core]
        blk[:, :, :QF, :QF] = q1 + q2
        blk[:, :, QF:, :QF] = (q1 - q2)[:, :, ::-1, :]
        blk[:, :, :QF, QF:] = (q3 + q4)[:, :, :, ::-1]
        blk[:, :, QF:, QF:] = (q3 - q4)[:, :, ::-1, ::-1]
    if _trace:
        return out, res
    return out

